# revision 4
# baseline (speedup 1.0000x reference)
"""LayerNorm-LSTM cell (nn_LSTMCell) Trainium2 Bass kernel.

Strategy: data-parallel over the batch dim — each of the 8 NeuronCores
processes 1024 of the 8192 batch rows with replicated weights.

Per-core kernel (B=1024 rows, I=H=1024, 4H=4096):
  gates = x @ W_xh + h @ W_hh (+ bias)          # TensorE, bf16 operands
  per-gate groupnorm (4 groups of 1024)          # bn_stats on PSUM + fused
  i,j,f,o activations                            #   scale/bias on ScalarE
  new_c = c*sig(f+1) + sig(i)*tanh(j)            # VectorE
  new_h = tanh(LN(new_c)) * sig(o)               # ScalarE+VectorE

Layout: batch rows on SBUF partitions everywhere.  x/h are transposed on
the host (marshaling) so the contraction dim I lands on partitions for the
matmul; x/h/W are host-cast to bf16 (PE streams 1 col/cycle regardless of
dtype, bf16 halves HBM traffic and SBUF footprint; c and outputs stay
fp32).  The gate dim is processed gate-at-a-time so each (gate, block)
pair's two [128,512] PSUM tiles are consumed (stats + fused activation)
straight out of PSUM with no raw staging in SBUF.
"""

import sys

if "/opt/trn_rl_repo" not in sys.path:
    sys.path.insert(0, "/opt/trn_rl_repo")

import ml_dtypes
import numpy as np

import concourse.bass as bass
import concourse.mybir as mybir
import concourse.tile as tile
from concourse.bass_utils import run_bass_kernel_spmd

P = 128
B, I, H = 8192, 1024, 1024
G4 = 4 * H
NCORES = 8
BC = B // NCORES          # 1024 batch rows per core
NB = BC // P              # 8 row blocks per core
KS = I // P               # 8 k-subtiles of the contraction dim
EPS = 1e-3
FORGET_BIAS = 1.0
BF16 = mybir.dt.bfloat16
F32 = mybir.dt.float32
AF = mybir.ActivationFunctionType

# ---------------------------------------------------------------------------
# Workaround: the walrus build in this container rejects TPB CTRL
# instructions carrying more than ONE semaphore wait ("Too many sync wait
# commands").  Split fat wait lists into single-wait NoOps on the same
# engine, inserted immediately before the instruction (semantics identical:
# all waits must hold before the instruction executes either way).
_TPB_ENGINES = None


def _split_fat_waits(nc, max_waits=1):
    global _TPB_ENGINES
    if _TPB_ENGINES is None:
        _TPB_ENGINES = {
            mybir.EngineType.PE,
            mybir.EngineType.Activation,
            mybir.EngineType.DVE,
            mybir.EngineType.Pool,
            mybir.EngineType.SP,
        }
    n = 0
    for func in nc.m.functions:
        for bb in func.blocks:
            out = []
            for ins in bb.instructions:
                si = getattr(ins, "sync_info", None)
                eng = getattr(ins, "engine", None)
                if (
                    si is not None
                    and si.on_wait
                    and len(si.on_wait) > max_waits
                    and eng in _TPB_ENGINES
                ):
                    waits = list(si.on_wait)
                    overflow, keep = waits[:-max_waits], waits[-max_waits:]
                    for cs in range(0, len(overflow), max_waits):
                        nop = mybir.InstNoOp(
                            name=f"{ins.name}-ws{cs}",
                            engine=eng,
                            sync_info=mybir.SyncInfo(
                                on_wait=overflow[cs : cs + max_waits], on_update=[]
                            ),
                            text_hint="waitsplit",
                        )
                        out.append(nop)
                        n += 1
                    si.on_wait = keep
                out.append(ins)
            bb.instructions = out
    return n


# ---------------------------------------------------------------------------


def _build(trivial):
    """Build the per-core Bass program.  `trivial` skips the (identity)
    groupnorm affine and the (zero) pre-norm bias."""
    nc = bass.Bass("TRN2", target_bir_lowering=False, debug=False, num_devices=NCORES)

    xT = nc.declare_dram_parameter("xT", [I, BC], BF16, isOutput=False).ap()
    hT = nc.declare_dram_parameter("hT", [I, BC], BF16, isOutput=False).ap()
    c_in = nc.declare_dram_parameter("c", [BC, H], F32, isOutput=False).ap()
    wxh = nc.declare_dram_parameter("Wxh", [I, G4], BF16, isOutput=False).ap()
    whh = nc.declare_dram_parameter("Whh", [I, G4], BF16, isOutput=False).ap()
    if not trivial:
        biasv = nc.declare_dram_parameter("biasv", [1, G4], BF16, isOutput=False).ap()
        g4v = nc.declare_dram_parameter("g4v", [1, G4], F32, isOutput=False).ap()
        b4v = nc.declare_dram_parameter("b4v", [1, G4], F32, isOutput=False).ap()
        gcv = nc.declare_dram_parameter("gcv", [1, H], F32, isOutput=False).ap()
        bcv = nc.declare_dram_parameter("bcv", [1, H], F32, isOutput=False).ap()
    new_h = nc.declare_dram_parameter("new_h", [BC, H], F32, isOutput=True).ap()
    new_c = nc.declare_dram_parameter("new_c", [BC, H], F32, isOutput=True).ap()

    xT_r = xT.rearrange("(ks p) b -> p ks b", p=P)
    hT_r = hT.rearrange("(ks p) b -> p ks b", p=P)
    wxh_r = wxh.rearrange("(ks p) n -> p ks n", p=P)
    whh_r = whh.rearrange("(ks p) n -> p ks n", p=P)

    with tile.TileContext(nc) as tc:
        with (
            tc.tile_pool(name="resx", bufs=1) as resx,
            tc.tile_pool(name="resh", bufs=1) as resh,
            tc.tile_pool(name="wp", bufs=3) as wp,
            tc.tile_pool(name="psum", bufs=8, space="PSUM") as psump,
            tc.tile_pool(name="acti", bufs=14) as actip,
            tc.tile_pool(name="cp", bufs=3) as cp,
            tc.tile_pool(name="ncp", bufs=3) as ncp,
            tc.tile_pool(name="nhp", bufs=3) as nhp,
            tc.tile_pool(name="stat", bufs=10) as statp,
            tc.tile_pool(name="small", bufs=24) as smallp,
            tc.tile_pool(name="singles", bufs=1) as singles,
            tc.tile_pool(name="gen", bufs=4) as genp,
        ):
            eps_t = singles.tile([P, 1], F32)
            nc.vector.memset(eps_t, EPS)

            if not trivial:
                ones_t = singles.tile([1, P], BF16)
                nc.vector.memset(ones_t, 1.0)
                bias_sb = singles.tile([1, G4], BF16)
                nc.sync.dma_start(out=bias_sb, in_=biasv[:])
                # replicate gamma/beta across all 128 partitions via DMA
                g4_sb = singles.tile([P, G4], F32)
                b4_sb = singles.tile([P, G4], F32)
                gc_sb = singles.tile([P, H], F32)
                bc_sb = singles.tile([P, H], F32)
                for vec, sb, width in (
                    (g4v, g4_sb, G4),
                    (b4v, b4_sb, G4),
                    (gcv, gc_sb, H),
                    (bcv, bc_sb, H),
                ):
                    bcast = bass.AP(
                        tensor=vec.tensor,
                        offset=vec.offset,
                        ap=[[0, P], vec.ap[1]],
                    )
                    nc.sync.dma_start(out=sb, in_=bcast)

            # resident transposed activations, bf16 [128, ks, 1024].
            # Per-k-subtile DMAs so the first matmuls only wait on chunk 0.
            xt_sb = resx.tile([P, KS, BC], BF16)
            ht_sb = resh.tile([P, KS, BC], BF16)
            for ks in range(KS):
                nc.sync.dma_start(out=xt_sb[:, ks, :], in_=xT_r[:, ks, :])
                nc.sync.dma_start(out=ht_sb[:, ks, :], in_=hT_r[:, ks, :])

            m1s = [None] * NB     # sig(i)*tanh(j), bf16 per block
            tclns = [None] * NB   # tanh(LN(new_c)), bf16 per block
            cbs = [None] * NB

            def stats_rstd_negmu(ps_pair, add_forget):
                """bn stats over the two 512-wide halves -> (rstd, bias) APs."""
                st = statp.tile([P, 2, 6], F32)
                nc.vector.bn_stats(out=st[:, 0, :], in_=ps_pair[0])
                nc.vector.bn_stats(out=st[:, 1, :], in_=ps_pair[1])
                mv = statp.tile([P, 2], F32)
                nc.vector.bn_aggr(out=mv, in_=st)
                mean, var = mv[:, 0:1], mv[:, 1:2]
                sd = smallp.tile([P, 1], F32)
                nc.scalar.activation(sd, var, AF.Sqrt, bias=eps_t, scale=1.0)
                rs = smallp.tile([P, 1], F32)
                nc.vector.reciprocal(rs, sd)
                nm = smallp.tile([P, 1], F32)
                nc.vector.tensor_mul(nm, mean, rs)
                if add_forget:
                    # bias = 1 - mean*rstd
                    nc.vector.tensor_scalar(
                        out=nm, in0=nm, scalar1=-1.0, scalar2=FORGET_BIAS,
                        op0=mybir.AluOpType.mult, op1=mybir.AluOpType.add,
                    )
                else:
                    nc.vector.tensor_scalar_mul(out=nm, in0=nm, scalar1=-1.0)
                return rs, nm

            for g in range(4):
                gc0 = g * H
                wx_sb = wp.tile([P, KS, H], BF16, tag="w")
                wh_sb = wp.tile([P, KS, H], BF16, tag="w")
                for ks in range(KS):
                    nc.sync.dma_start(
                        out=wx_sb[:, ks, :], in_=wxh_r[:, ks, gc0 : gc0 + H]
                    )
                    nc.sync.dma_start(
                        out=wh_sb[:, ks, :], in_=whh_r[:, ks, gc0 : gc0 + H]
                    )
                func = AF.Tanh if g == 1 else AF.Sigmoid

                for b in range(NB):
                    b0 = b * P
                    pss = []
                    for half in range(2):
                        hc = half * 512
                        ps = psump.tile([P, 512], F32, tag="ps")
                        for ks in range(KS):
                            nc.tensor.matmul(
                                ps,
                                lhsT=xt_sb[:, ks, b0 : b0 + P],
                                rhs=wx_sb[:, ks, hc : hc + 512],
                                start=(ks == 0),
                                stop=False,
                            )
                        last = KS - 1
                        for ks in range(KS):
                            nc.tensor.matmul(
                                ps,
                                lhsT=ht_sb[:, ks, b0 : b0 + P],
                                rhs=wh_sb[:, ks, hc : hc + 512],
                                start=False,
                                stop=(trivial and ks == last),
                            )
                        if not trivial:
                            nc.tensor.matmul(
                                ps,
                                lhsT=ones_t,
                                rhs=bias_sb[:, gc0 + hc : gc0 + hc + 512],
                                start=False,
                                stop=True,
                            )
                        pss.append(ps)

                    rs, nm = stats_rstd_negmu(pss, add_forget=(trivial and g == 2))

                    act = actip.tile([P, H], BF16, tag="act")
                    for half in range(2):
                        hc = half * 512
                        if trivial:
                            nc.scalar.activation(
                                act[:, hc : hc + 512], pss[half], func,
                                bias=nm, scale=rs,
                            )
                        else:
                            t = genp.tile([P, 512], F32, tag="gtmp")
                            # (x*r) + (-mu*r) == (x-mu)*r
                            nc.vector.tensor_scalar(
                                out=t, in0=pss[half],
                                scalar1=rs, scalar2=nm,
                                op0=mybir.AluOpType.mult, op1=mybir.AluOpType.add,
                            )
                            nc.vector.tensor_mul(
                                t, t, g4_sb[:, gc0 + hc : gc0 + hc + 512]
                            )
                            nc.vector.tensor_add(
                                t, t, b4_sb[:, gc0 + hc : gc0 + hc + 512]
                            )
                            nc.scalar.activation(
                                act[:, hc : hc + 512], t, func,
                                bias=(FORGET_BIAS if g == 2 else 0.0), scale=1.0,
                            )

                    if g == 0:
                        m1s[b] = act
                        # prefetch c for this block (used in gate-f phase)
                        cb = cp.tile([P, H], F32, tag="c")
                        nc.sync.dma_start(out=cb, in_=c_in[b0 : b0 + P, :])
                        cbs[b] = cb
                    elif g == 1:
                        # m1 = sig(i) * tanh(j), in place over sig(i)
                        nc.vector.tensor_mul(m1s[b], m1s[b], act)
                    elif g == 2:
                        ncv = ncp.tile([P, H], F32, tag="nc")
                        nc.vector.tensor_mul(ncv, cbs[b], act)
                        nc.vector.tensor_add(ncv, ncv, m1s[b])
                        nc.gpsimd.dma_start(out=new_c[b0 : b0 + P, :], in_=ncv)
                        # LN over new_c, then tanh
                        st2 = statp.tile([P, 2, 6], F32)
                        nc.vector.bn_stats(out=st2[:, 0, :], in_=ncv[:, 0:512])
                        nc.vector.bn_stats(out=st2[:, 1, :], in_=ncv[:, 512:1024])
                        mv2 = statp.tile([P, 2], F32)
                        nc.vector.bn_aggr(out=mv2, in_=st2)
                        sd2 = smallp.tile([P, 1], F32)
                        nc.scalar.activation(
                            sd2, mv2[:, 1:2], AF.Sqrt, bias=eps_t, scale=1.0
                        )
                        rs2 = smallp.tile([P, 1], F32)
                        nc.vector.reciprocal(rs2, sd2)
                        nm2 = smallp.tile([P, 1], F32)
                        nc.vector.tensor_mul(nm2, mv2[:, 0:1], rs2)
                        nc.vector.tensor_scalar_mul(out=nm2, in0=nm2, scalar1=-1.0)
                        tcl = actip.tile([P, H], BF16, tag="act")
                        if trivial:
                            nc.scalar.activation(
                                tcl, ncv, AF.Tanh, bias=nm2, scale=rs2
                            )
                        else:
                            t2 = genp.tile([P, H], F32, tag="gtmp2")
                            nc.vector.tensor_scalar(
                                out=t2, in0=ncv, scalar1=rs2, scalar2=nm2,
                                op0=mybir.AluOpType.mult, op1=mybir.AluOpType.add,
                            )
                            nc.vector.tensor_mul(t2, t2, gc_sb)
                            nc.vector.tensor_add(t2, t2, bc_sb)
                            nc.scalar.activation(tcl, t2, AF.Tanh, bias=0.0, scale=1.0)
                        tclns[b] = tcl
                    else:
                        nh = nhp.tile([P, H], F32, tag="nh")
                        nc.vector.tensor_mul(nh, tclns[b], act)
                        nc.gpsimd.dma_start(out=new_h[b0 : b0 + P, :], in_=nh)

    _split_fat_waits(nc)
    return nc


_CACHE = {}
LAST_RESULTS = None


def kernel(x, c, h, W_xh, W_hh, bias, ln_gamma, ln_beta, ln_c_gamma, ln_c_beta,
           _trace=False):
    x = np.asarray(x, np.float32)
    c = np.asarray(c, np.float32)
    h = np.asarray(h, np.float32)
    W_xh = np.asarray(W_xh, np.float32)
    W_hh = np.asarray(W_hh, np.float32)
    bias = np.asarray(bias, np.float32)
    ln_gamma = np.asarray(ln_gamma, np.float32)
    ln_beta = np.asarray(ln_beta, np.float32)
    ln_c_gamma = np.asarray(ln_c_gamma, np.float32)
    ln_c_beta = np.asarray(ln_c_beta, np.float32)

    trivial = bool(
        (bias == 0).all()
        and (ln_gamma == 1).all()
        and (ln_beta == 0).all()
        and (ln_c_gamma == 1).all()
        and (ln_c_beta == 0).all()
    )

    if trivial not in _CACHE:
        _CACHE[trivial] = _build(trivial)
    nc = _CACHE[trivial]

    bf = ml_dtypes.bfloat16
    xT = np.ascontiguousarray(x.T).astype(bf)      # [I, B]
    hT = np.ascontiguousarray(h.T).astype(bf)
    wx16 = W_xh.astype(bf)
    wh16 = W_hh.astype(bf)

    in_maps = []
    for i in range(NCORES):
        s = i * BC
        m = {
            "xT": np.ascontiguousarray(xT[:, s : s + BC]),
            "hT": np.ascontiguousarray(hT[:, s : s + BC]),
            "c": np.ascontiguousarray(c[s : s + BC]),
            "Wxh": wx16,
            "Whh": wh16,
        }
        if not trivial:
            m["biasv"] = bias.astype(bf).reshape(1, G4)
            m["g4v"] = ln_gamma.reshape(1, G4)
            m["b4v"] = ln_beta.reshape(1, G4)
            m["gcv"] = ln_c_gamma.reshape(1, H)
            m["bcv"] = ln_c_beta.reshape(1, H)
        in_maps.append(m)

    res = run_bass_kernel_spmd(nc, in_maps, list(range(NCORES)), trace=_trace)
    global LAST_RESULTS
    LAST_RESULTS = res

    out_h = np.concatenate([res.results[i]["new_h"] for i in range(NCORES)], axis=0)
    out_c = np.concatenate([res.results[i]["new_c"] for i in range(NCORES)], axis=0)
    return out_h, out_c


# revision 6
# speedup vs baseline: 1.0204x; 1.0204x over previous
"""LayerNorm-LSTM cell (nn_LSTMCell) Trainium2 Bass kernel.

Strategy: data-parallel over the batch dim — each of the 8 NeuronCores
processes 1024 of the 8192 batch rows with replicated weights.

Per-core kernel (B=1024 rows, I=H=1024, 4H=4096):
  gates = x @ W_xh + h @ W_hh (+ bias)          # TensorE, bf16 operands
  per-gate groupnorm (4 groups of 1024)          # bn_stats on PSUM + fused
  i,j,f,o activations                            #   scale/bias on ScalarE
  new_c = c*sig(f+1) + sig(i)*tanh(j)            # VectorE
  new_h = tanh(LN(new_c)) * sig(o)               # ScalarE+VectorE

Layout: batch rows on SBUF partitions everywhere.  x/h are transposed on
the host (marshaling) so the contraction dim I lands on partitions for the
matmul; x/h/W are host-cast to bf16 (PE streams 1 col/cycle regardless of
dtype, bf16 halves HBM traffic and SBUF footprint; c and outputs stay
fp32).  The gate dim is processed gate-at-a-time so each (gate, block)
pair's two [128,512] PSUM tiles are consumed (stats + fused activation)
straight out of PSUM with no raw staging in SBUF.
"""

import sys

if "/opt/trn_rl_repo" not in sys.path:
    sys.path.insert(0, "/opt/trn_rl_repo")

import ml_dtypes
import numpy as np

import concourse.bass as bass
import concourse.mybir as mybir
import concourse.tile as tile
from concourse.bass_utils import run_bass_kernel_spmd

P = 128
B, I, H = 8192, 1024, 1024
G4 = 4 * H
NCORES = 8
BC = B // NCORES          # 1024 batch rows per core
NB = BC // P              # 8 row blocks per core
KS = I // P               # 8 k-subtiles of the contraction dim
EPS = 1e-3
FORGET_BIAS = 1.0
BF16 = mybir.dt.bfloat16
F32 = mybir.dt.float32
AF = mybir.ActivationFunctionType

# ---------------------------------------------------------------------------
# Workaround: the walrus build in this container rejects TPB CTRL
# instructions carrying more than ONE semaphore wait ("Too many sync wait
# commands").  Split fat wait lists into single-wait NoOps on the same
# engine, inserted immediately before the instruction (semantics identical:
# all waits must hold before the instruction executes either way).
_TPB_ENGINES = None


def _split_fat_waits(nc, max_waits=1):
    global _TPB_ENGINES
    if _TPB_ENGINES is None:
        _TPB_ENGINES = {
            mybir.EngineType.PE,
            mybir.EngineType.Activation,
            mybir.EngineType.DVE,
            mybir.EngineType.Pool,
            mybir.EngineType.SP,
        }
    n = 0
    for func in nc.m.functions:
        for bb in func.blocks:
            out = []
            for ins in bb.instructions:
                si = getattr(ins, "sync_info", None)
                eng = getattr(ins, "engine", None)
                if (
                    si is not None
                    and si.on_wait
                    and len(si.on_wait) > max_waits
                    and eng in _TPB_ENGINES
                ):
                    waits = list(si.on_wait)
                    overflow, keep = waits[:-max_waits], waits[-max_waits:]
                    for cs in range(0, len(overflow), max_waits):
                        nop = mybir.InstNoOp(
                            name=f"{ins.name}-ws{cs}",
                            engine=eng,
                            sync_info=mybir.SyncInfo(
                                on_wait=overflow[cs : cs + max_waits], on_update=[]
                            ),
                            text_hint="waitsplit",
                        )
                        out.append(nop)
                        n += 1
                    si.on_wait = keep
                out.append(ins)
            bb.instructions = out
    return n


# ---------------------------------------------------------------------------


def _build(trivial):
    """Build the per-core Bass program.  `trivial` skips the (identity)
    groupnorm affine and the (zero) pre-norm bias."""
    nc = bass.Bass("TRN2", target_bir_lowering=False, debug=False, num_devices=NCORES)

    xT = nc.declare_dram_parameter("xT", [I, BC], BF16, isOutput=False).ap()
    hT = nc.declare_dram_parameter("hT", [I, BC], BF16, isOutput=False).ap()
    c_in = nc.declare_dram_parameter("c", [BC, H], F32, isOutput=False).ap()
    wxh = nc.declare_dram_parameter("Wxh", [I, G4], BF16, isOutput=False).ap()
    whh = nc.declare_dram_parameter("Whh", [I, G4], BF16, isOutput=False).ap()
    if not trivial:
        biasv = nc.declare_dram_parameter("biasv", [1, G4], BF16, isOutput=False).ap()
        g4v = nc.declare_dram_parameter("g4v", [1, G4], F32, isOutput=False).ap()
        b4v = nc.declare_dram_parameter("b4v", [1, G4], F32, isOutput=False).ap()
        gcv = nc.declare_dram_parameter("gcv", [1, H], F32, isOutput=False).ap()
        bcv = nc.declare_dram_parameter("bcv", [1, H], F32, isOutput=False).ap()
    new_h = nc.declare_dram_parameter("new_h", [BC, H], F32, isOutput=True).ap()
    new_c = nc.declare_dram_parameter("new_c", [BC, H], F32, isOutput=True).ap()

    xT_r = xT.rearrange("(ks p) b -> p ks b", p=P)
    hT_r = hT.rearrange("(ks p) b -> p ks b", p=P)
    wxh_r = wxh.rearrange("(ks p) n -> p ks n", p=P)
    whh_r = whh.rearrange("(ks p) n -> p ks n", p=P)

    with tile.TileContext(nc) as tc:
        with (
            tc.tile_pool(name="resx", bufs=1) as resx,
            tc.tile_pool(name="resh", bufs=1) as resh,
            tc.tile_pool(name="wp", bufs=3) as wp,
            tc.tile_pool(name="psum", bufs=8, space="PSUM") as psump,
            tc.tile_pool(name="acti", bufs=14) as actip,
            tc.tile_pool(name="cp", bufs=3) as cp,
            tc.tile_pool(name="ncp", bufs=3) as ncp,
            tc.tile_pool(name="nhp", bufs=3) as nhp,
            tc.tile_pool(name="stat", bufs=10) as statp,
            tc.tile_pool(name="small", bufs=24) as smallp,
            tc.tile_pool(name="singles", bufs=1) as singles,
            tc.tile_pool(name="gen", bufs=4) as genp,
        ):
            eps_t = singles.tile([P, 1], F32)
            nc.vector.memset(eps_t, EPS)

            if not trivial:
                ones_t = singles.tile([1, P], BF16)
                nc.vector.memset(ones_t, 1.0)
                bias_sb = singles.tile([1, G4], BF16)
                nc.sync.dma_start(out=bias_sb, in_=biasv[:])
                # replicate gamma/beta across all 128 partitions via DMA
                g4_sb = singles.tile([P, G4], F32)
                b4_sb = singles.tile([P, G4], F32)
                gc_sb = singles.tile([P, H], F32)
                bc_sb = singles.tile([P, H], F32)
                for vec, sb, width in (
                    (g4v, g4_sb, G4),
                    (b4v, b4_sb, G4),
                    (gcv, gc_sb, H),
                    (bcv, bc_sb, H),
                ):
                    bcast = bass.AP(
                        tensor=vec.tensor,
                        offset=vec.offset,
                        ap=[[0, P], vec.ap[1]],
                    )
                    nc.sync.dma_start(out=sb, in_=bcast)

            # resident transposed activations, bf16 [128, ks, 1024].
            # Per-k-subtile DMAs so the first matmuls only wait on chunk 0,
            # interleaved with gate-0's W chunks (matmul ks consumes only
            # xt[ks]+wx[ks], so issue them pairwise: the PE unblocks after
            # two chunks instead of after the whole 8MB startup load).
            xt_sb = resx.tile([P, KS, BC], BF16)
            ht_sb = resh.tile([P, KS, BC], BF16)
            wx0_sb = wp.tile([P, KS, H], BF16, tag="w")
            wh0_sb = wp.tile([P, KS, H], BF16, tag="w")
            for ks in range(KS):
                nc.sync.dma_start(out=wx0_sb[:, ks, :], in_=wxh_r[:, ks, 0:H])
                nc.sync.dma_start(out=xt_sb[:, ks, :], in_=xT_r[:, ks, :])
            for ks in range(KS):
                nc.sync.dma_start(out=wh0_sb[:, ks, :], in_=whh_r[:, ks, 0:H])
                nc.sync.dma_start(out=ht_sb[:, ks, :], in_=hT_r[:, ks, :])

            m1s = [None] * NB     # sig(i)*tanh(j), bf16 per block
            tclns = [None] * NB   # tanh(LN(new_c)), bf16 per block
            cbs = [None] * NB

            def stats_rstd_negmu(ps_pair, add_forget):
                """bn stats over the two 512-wide halves -> (rstd, bias) APs."""
                st = statp.tile([P, 2, 6], F32)
                nc.vector.bn_stats(out=st[:, 0, :], in_=ps_pair[0])
                nc.vector.bn_stats(out=st[:, 1, :], in_=ps_pair[1])
                mv = statp.tile([P, 2], F32)
                nc.vector.bn_aggr(out=mv, in_=st)
                mean, var = mv[:, 0:1], mv[:, 1:2]
                sd = smallp.tile([P, 1], F32)
                nc.scalar.activation(sd, var, AF.Sqrt, bias=eps_t, scale=1.0)
                rs = smallp.tile([P, 1], F32)
                nc.vector.reciprocal(rs, sd)
                nm = smallp.tile([P, 1], F32)
                nc.vector.tensor_mul(nm, mean, rs)
                if add_forget:
                    # bias = 1 - mean*rstd
                    nc.vector.tensor_scalar(
                        out=nm, in0=nm, scalar1=-1.0, scalar2=FORGET_BIAS,
                        op0=mybir.AluOpType.mult, op1=mybir.AluOpType.add,
                    )
                else:
                    nc.vector.tensor_scalar_mul(out=nm, in0=nm, scalar1=-1.0)
                return rs, nm

            for g in range(4):
                gc0 = g * H
                if g == 0:
                    wx_sb, wh_sb = wx0_sb, wh0_sb
                else:
                    wx_sb = wp.tile([P, KS, H], BF16, tag="w")
                    wh_sb = wp.tile([P, KS, H], BF16, tag="w")
                    for ks in range(KS):
                        nc.sync.dma_start(
                            out=wx_sb[:, ks, :], in_=wxh_r[:, ks, gc0 : gc0 + H]
                        )
                        nc.sync.dma_start(
                            out=wh_sb[:, ks, :], in_=whh_r[:, ks, gc0 : gc0 + H]
                        )
                func = AF.Tanh if g == 1 else AF.Sigmoid

                for b in range(NB):
                    b0 = b * P
                    pss = []
                    for half in range(2):
                        hc = half * 512
                        ps = psump.tile([P, 512], F32, tag="ps")
                        for ks in range(KS):
                            nc.tensor.matmul(
                                ps,
                                lhsT=xt_sb[:, ks, b0 : b0 + P],
                                rhs=wx_sb[:, ks, hc : hc + 512],
                                start=(ks == 0),
                                stop=False,
                            )
                        last = KS - 1
                        for ks in range(KS):
                            nc.tensor.matmul(
                                ps,
                                lhsT=ht_sb[:, ks, b0 : b0 + P],
                                rhs=wh_sb[:, ks, hc : hc + 512],
                                start=False,
                                stop=(trivial and ks == last),
                            )
                        if not trivial:
                            nc.tensor.matmul(
                                ps,
                                lhsT=ones_t,
                                rhs=bias_sb[:, gc0 + hc : gc0 + hc + 512],
                                start=False,
                                stop=True,
                            )
                        pss.append(ps)

                    rs, nm = stats_rstd_negmu(pss, add_forget=(trivial and g == 2))

                    act = actip.tile([P, H], BF16, tag="act")
                    for half in range(2):
                        hc = half * 512
                        if trivial:
                            nc.scalar.activation(
                                act[:, hc : hc + 512], pss[half], func,
                                bias=nm, scale=rs,
                            )
                        else:
                            t = genp.tile([P, 512], F32, tag="gtmp")
                            # (x*r) + (-mu*r) == (x-mu)*r
                            nc.vector.tensor_scalar(
                                out=t, in0=pss[half],
                                scalar1=rs, scalar2=nm,
                                op0=mybir.AluOpType.mult, op1=mybir.AluOpType.add,
                            )
                            nc.vector.tensor_mul(
                                t, t, g4_sb[:, gc0 + hc : gc0 + hc + 512]
                            )
                            nc.vector.tensor_add(
                                t, t, b4_sb[:, gc0 + hc : gc0 + hc + 512]
                            )
                            nc.scalar.activation(
                                act[:, hc : hc + 512], t, func,
                                bias=(FORGET_BIAS if g == 2 else 0.0), scale=1.0,
                            )

                    if g == 0:
                        m1s[b] = act
                        # prefetch c for this block (used in gate-f phase)
                        cb = cp.tile([P, H], F32, tag="c")
                        nc.sync.dma_start(out=cb, in_=c_in[b0 : b0 + P, :])
                        cbs[b] = cb
                    elif g == 1:
                        # m1 = sig(i) * tanh(j), in place over sig(i)
                        nc.vector.tensor_mul(m1s[b], m1s[b], act)
                    elif g == 2:
                        ncv = ncp.tile([P, H], F32, tag="nc")
                        nc.vector.tensor_mul(ncv, cbs[b], act)
                        nc.vector.tensor_add(ncv, ncv, m1s[b])
                        nc.gpsimd.dma_start(out=new_c[b0 : b0 + P, :], in_=ncv)
                        # LN over new_c, then tanh
                        st2 = statp.tile([P, 2, 6], F32)
                        nc.vector.bn_stats(out=st2[:, 0, :], in_=ncv[:, 0:512])
                        nc.vector.bn_stats(out=st2[:, 1, :], in_=ncv[:, 512:1024])
                        mv2 = statp.tile([P, 2], F32)
                        nc.vector.bn_aggr(out=mv2, in_=st2)
                        sd2 = smallp.tile([P, 1], F32)
                        nc.scalar.activation(
                            sd2, mv2[:, 1:2], AF.Sqrt, bias=eps_t, scale=1.0
                        )
                        rs2 = smallp.tile([P, 1], F32)
                        nc.vector.reciprocal(rs2, sd2)
                        nm2 = smallp.tile([P, 1], F32)
                        nc.vector.tensor_mul(nm2, mv2[:, 0:1], rs2)
                        nc.vector.tensor_scalar_mul(out=nm2, in0=nm2, scalar1=-1.0)
                        tcl = actip.tile([P, H], BF16, tag="act")
                        if trivial:
                            nc.scalar.activation(
                                tcl, ncv, AF.Tanh, bias=nm2, scale=rs2
                            )
                        else:
                            t2 = genp.tile([P, H], F32, tag="gtmp2")
                            nc.vector.tensor_scalar(
                                out=t2, in0=ncv, scalar1=rs2, scalar2=nm2,
                                op0=mybir.AluOpType.mult, op1=mybir.AluOpType.add,
                            )
                            nc.vector.tensor_mul(t2, t2, gc_sb)
                            nc.vector.tensor_add(t2, t2, bc_sb)
                            nc.scalar.activation(tcl, t2, AF.Tanh, bias=0.0, scale=1.0)
                        tclns[b] = tcl
                    else:
                        nh = nhp.tile([P, H], F32, tag="nh")
                        nc.vector.tensor_mul(nh, tclns[b], act)
                        nc.gpsimd.dma_start(out=new_h[b0 : b0 + P, :], in_=nh)

    _split_fat_waits(nc)
    return nc


_CACHE = {}
LAST_RESULTS = None


def kernel(x, c, h, W_xh, W_hh, bias, ln_gamma, ln_beta, ln_c_gamma, ln_c_beta,
           _trace=False):
    x = np.asarray(x, np.float32)
    c = np.asarray(c, np.float32)
    h = np.asarray(h, np.float32)
    W_xh = np.asarray(W_xh, np.float32)
    W_hh = np.asarray(W_hh, np.float32)
    bias = np.asarray(bias, np.float32)
    ln_gamma = np.asarray(ln_gamma, np.float32)
    ln_beta = np.asarray(ln_beta, np.float32)
    ln_c_gamma = np.asarray(ln_c_gamma, np.float32)
    ln_c_beta = np.asarray(ln_c_beta, np.float32)

    trivial = bool(
        (bias == 0).all()
        and (ln_gamma == 1).all()
        and (ln_beta == 0).all()
        and (ln_c_gamma == 1).all()
        and (ln_c_beta == 0).all()
    )

    if trivial not in _CACHE:
        _CACHE[trivial] = _build(trivial)
    nc = _CACHE[trivial]

    bf = ml_dtypes.bfloat16
    xT = np.ascontiguousarray(x.T).astype(bf)      # [I, B]
    hT = np.ascontiguousarray(h.T).astype(bf)
    wx16 = W_xh.astype(bf)
    wh16 = W_hh.astype(bf)

    in_maps = []
    for i in range(NCORES):
        s = i * BC
        m = {
            "xT": np.ascontiguousarray(xT[:, s : s + BC]),
            "hT": np.ascontiguousarray(hT[:, s : s + BC]),
            "c": np.ascontiguousarray(c[s : s + BC]),
            "Wxh": wx16,
            "Whh": wh16,
        }
        if not trivial:
            m["biasv"] = bias.astype(bf).reshape(1, G4)
            m["g4v"] = ln_gamma.reshape(1, G4)
            m["b4v"] = ln_beta.reshape(1, G4)
            m["gcv"] = ln_c_gamma.reshape(1, H)
            m["bcv"] = ln_c_beta.reshape(1, H)
        in_maps.append(m)

    res = run_bass_kernel_spmd(nc, in_maps, list(range(NCORES)), trace=_trace)
    global LAST_RESULTS
    LAST_RESULTS = res

    out_h = np.concatenate([res.results[i]["new_h"] for i in range(NCORES)], axis=0)
    out_c = np.concatenate([res.results[i]["new_c"] for i in range(NCORES)], axis=0)
    return out_h, out_c
